# revision 1
# baseline (speedup 1.0000x reference)
"""Criss-cross attention (CCNet) kernel for 8 TRN2 NeuronCores.

Data-parallel over batch N=8: one image per core. Per image (512ch, 96x96):
  t/f = 1x1 conv to 64ch; g = 1x1 conv to 512ch
  row/col affinities -> softmax over 191 (96 row + 95 col, col diag excluded)
  weighted row/col aggregation of g -> inc 1x1 conv -> residual add.

All matmuls bf16 operands with f32 PSUM accumulation. Weight transposes and
bf16 casts are done on host (numpy) - they are kernel inputs.

v2 (vs v1 baseline):
  - t/f conv deduped: one [t;f] stream; affinity matmuls use explicit
    tile_position=(0,0) so F (partitions 64:128) pairs with T (0:64).
  - softmax reciprocal on DVE (reciprocal_approx_fast) instead of Ln+Exp
    on Activation - kills all activation-table reloads.
  - affinity exp in 4-row chunks (one Exp per chunk), mask in one Pool op.
  - col-pass g-tile PSUM->SBUF casts on Pool, row-pass on Activation.
  - row-denominator + Wr normalize fused into the row pass (pipelined).
  - residual add reads bf16 Xbf from SBUF (x_f32 stream dropped).
"""

import sys

sys.path.insert(0, "/opt/trn_rl_repo")

from contextlib import ExitStack

import numpy as np
import ml_dtypes

import concourse.bass as bass
import concourse.bacc as bacc
import concourse.tile as tile
from concourse import mybir
from concourse.bass_utils import run_bass_kernel_spmd

BF16 = mybir.dt.bfloat16
F32 = mybir.dt.float32
AF = mybir.ActivationFunctionType

N, C_IN, C_INNER, C_OUT, H, W = 8, 512, 64, 512, 96, 96
HW = H * W  # 9216
KC = C_IN // 128  # 4 contraction chunks

_cache = {}


def build_program():
    nc = bacc.Bacc()

    # ---- DRAM I/O ----
    xbf_d = nc.dram_tensor("x_bf", (128, KC, HW), BF16, kind="ExternalInput")
    tfw_d = nc.dram_tensor("tf_wT", (128, KC, 128), BF16, kind="ExternalInput")
    gw_d = nc.dram_tensor("g_wT", (128, KC, C_OUT), BF16, kind="ExternalInput")
    incw_d = nc.dram_tensor("inc_wT", (128, KC, C_IN), BF16, kind="ExternalInput")
    tfb_d = nc.dram_tensor("tf_b", (128, 1), F32, kind="ExternalInput")
    combb_d = nc.dram_tensor("comb_b", (128, KC), F32, kind="ExternalInput")
    mask_d = nc.dram_tensor("mask", (96, 96), BF16, kind="ExternalInput")
    ones96b_d = nc.dram_tensor("ones96b", (96, 128), BF16, kind="ExternalInput")
    iden_d = nc.dram_tensor("iden128", (128, 128), BF16, kind="ExternalInput")
    out_d = nc.dram_tensor("out", (KC, 128, HW), BF16, kind="ExternalOutput")

    with ExitStack() as ctx:
        tc = ctx.enter_context(tile.TileContext(nc))
        p0 = ctx.enter_context(tc.tile_pool(name="p0", bufs=1))

        # ---- persistent tiles ----
        Xbf = p0.tile([128, KC, H, W], BF16)  # channel-major image, bf16
        ones96b = p0.tile([96, 128], BF16)
        mask = p0.tile([96, 96], BF16)
        gw = p0.tile([128, KC, C_OUT], BF16)
        iden = p0.tile([128, 128], BF16)

        nc.sync.dma_start(out=iden, in_=iden_d[:])
        nc.sync.dma_start(out=ones96b, in_=ones96b_d[:])
        nc.sync.dma_start(out=mask, in_=mask_d[:])
        nc.sync.dma_start(out=gw, in_=gw_d[:])
        xv = xbf_d[:].rearrange("p a (h w) -> p a h w", h=H)

        # TF (phase 1-2) and U (phases 3-4) share one big slot: disjoint
        # lifetimes.  Plane 0 = [t; f] stacked from the conv; plane 1
        # partitions 0:64 = copy of f re-based to partition 0 (matmul requires
        # both operands at the same partition base; DMA does the re-base).
        TF = p0.tile([128, 2, H, W], BF16, tag="big", name="TF")
        T = TF[0:64, 0]
        F = TF[0:64, 1]

        with tc.tile_pool(name="pwr", bufs=1) as pwr:
            # exp(affinity) buffers: Wr[i, y, x] (row), Wc[j, x, y] (col)
            Wr = pwr.tile([96, H, W], BF16)
            with tc.tile_pool(name="pwc", bufs=1) as pwc:
                Wc = pwc.tile([96, W, H], BF16)

                # ---- phase 1: t/f conv ----
                with tc.tile_pool(name="pe", bufs=1) as pe, \
                     tc.tile_pool(name="pe_ps", bufs=3, space="PSUM") as pe_ps, \
                     tc.tile_pool(name="ptf_ps", bufs=2, space="PSUM") as ptf_ps:
                    tfw = pe.tile([128, KC, 128], BF16)
                    tfb = pe.tile([128, 1], F32)
                    nc.sync.dma_start(out=tfw, in_=tfw_d[:])
                    nc.sync.dma_start(out=tfb, in_=tfb_d[:])
                    for q in range(8):
                        nc.sync.dma_start(
                            out=Xbf[:, :, q * 12:(q + 1) * 12, :],
                            in_=xv[:, :, q * 12:(q + 1) * 12, :])

                    Xflat = Xbf.rearrange("p a h w -> p a (h w)")
                    TFflat = TF.rearrange("p c h w -> p c (h w)")
                    for b in range(HW // 512):
                        sl = slice(b * 512, (b + 1) * 512)
                        pst = ptf_ps.tile([128, 512], F32, tag="pt")
                        for k in range(KC):
                            nc.tensor.matmul(
                                pst, tfw[:, k, :], Xflat[:, k, sl],
                                start=(k == 0), stop=(k == KC - 1))
                        nc.vector.tensor_scalar_add(TFflat[:, 0, sl], pst,
                                                    tfb)
                        # re-base f to partitions 0:64 (plane 1) for matmul
                        if b % 3 == 2:
                            sl3 = slice((b - 2) * 512, (b + 1) * 512)
                            nc.sync.dma_start(out=TFflat[0:64, 1, sl3],
                                              in_=TFflat[64:128, 0, sl3])

                    # ---- phase 2: affinities + exp ----
                    # 8 rows per chunk; psum tile is [96, 2, 512] so each
                    # 4-row matmul group stays inside one 2KB PSUM bank while
                    # a single Exp covers all 8 rows (amortizes Act overhead).
                    # row: E[i, x] = sum_c f[c,y,i] t[c,y,x]
                    for y0 in range(0, H, 8):
                        ps = pe_ps.tile([96, 2, 512], F32, tag="pe")
                        for r in range(8):
                            nc.tensor.matmul(
                                ps[:, r // 4, (r % 4) * 96:(r % 4) * 96 + 96],
                                F[:, y0 + r, :], T[:, y0 + r, :],
                                start=True, stop=True)
                        nc.scalar.activation(
                            Wr[:, y0:y0 + 8, :].rearrange(
                                "i (a b) w -> i a (b w)", a=2),
                            ps[:, :, 0:384], AF.Exp)
                    # col: E[j, y] = sum_c f[c,j,x] t[c,y,x]; kill j==y
                    mb8 = bass.AP(tensor=mask.tensor, offset=mask.offset,
                                  ap=[mask.ap[0], [0, 8], mask.ap[1]])
                    for x0 in range(0, W, 8):
                        ps = pe_ps.tile([96, 2, 512], F32, tag="pe")
                        for r in range(8):
                            nc.tensor.matmul(
                                ps[:, r // 4, (r % 4) * 96:(r % 4) * 96 + 96],
                                F[:, :, x0 + r], T[:, :, x0 + r],
                                start=True, stop=True)
                        wcs = Wc[:, x0:x0 + 8, :]
                        nc.scalar.activation(
                            wcs.rearrange("j (a b) y -> j a (b y)", a=2),
                            ps[:, :, 0:384], AF.Exp)
                        nc.gpsimd.tensor_mul(wcs, wcs, mb8)

                WrT = Wr.rearrange("i h w -> i w h")
                WcT = Wc.rearrange("j x y -> j y x")
                Wrflat = Wr.rearrange("p h w -> p (h w)")

                # ---- phase 3a: col pass (first writer of U) ----
                # g is computed once here; each per-column g tile is also
                # spilled to a DRAM pool tile Gd[x, y, c] and read back
                # transposed (Gd[:, y, :]) by the row pass - the DMA does the
                # pixel-axis transpose that PE recompute used to pay for.
                U = p0.tile([128, KC, H, W], BF16, tag="big", name="U")
                pgd = ctx.enter_context(tc.tile_pool(name="pgd", bufs=1,
                                                     space="DRAM"))
                Gd = pgd.tile([96, 96, C_OUT], BF16)  # [y, x, c]
                with tc.tile_pool(name="pu1", bufs=4) as pu1, \
                     tc.tile_pool(name="pg_ps1", bufs=3, space="PSUM") as pg_ps1, \
                     tc.tile_pool(name="pd_ps1", bufs=2, space="PSUM") as pd_ps1, \
                     tc.tile_pool(name="pu_ps1", bufs=3, space="PSUM") as pu_ps1:
                    # software pipeline: g-conv runs LA blocks ahead of the
                    # aggregation so PE never stalls on the Wc-dependent
                    # denominator matmuls while affinity exps drain on Act.
                    # The aggregation re-reads g from Gd (tile-tracked RAW)
                    # so the g-conv SBUF staging tiles recycle immediately.
                    def emit_g(x0):
                        gtb = pu1.tile([96, 4, C_OUT], BF16, tag="gt", bufs=2)
                        for r in range(4):
                            psg = pg_ps1.tile([96, C_OUT], F32, tag="pg")
                            for k in range(KC):
                                nc.tensor.matmul(psg, Xbf[:, k, :, x0 + r],
                                                 gw[:, k, :],
                                                 start=(k == 0), stop=(k == KC - 1))
                            if r % 2 == 0:
                                nc.vector.tensor_copy(gtb[:, r, :], psg)
                            else:
                                nc.scalar.activation(gtb[:, r, :], psg, AF.Copy)
                        nc.sync.dma_start(out=Gd[:, x0:x0 + 4, :], in_=gtb)

                    def emit_agg(x0, tail):
                        xs = slice(x0, x0 + 4)
                        psd = pd_ps1.tile([128, 4, 96], F32, tag="pd")
                        nc.tensor.matmul(psd, ones96b, Wc[:, xs, :],
                                         start=True, stop=False)
                        nc.tensor.matmul(psd, ones96b, WrT[:, xs, :],
                                         start=False, stop=True)
                        rr = pu1.tile([128, 4, 96], F32, tag="rr", bufs=2)
                        nc.vector.reciprocal_approx_fast(rr, psd)
                        rrT = rr.rearrange("p x y -> p y x")
                        if tail:
                            # tail blocks run with PE idle: fold 1/D into a
                            # normalized Wc copy on Pool (raw Wc preserved for
                            # the row-pass denominators) so the U write is a
                            # plain copy split across DVE and Act.
                            wcn = pu1.tile([96, 4, 96], BF16, tag="wcn", bufs=3)
                            nc.gpsimd.tensor_mul(wcn, Wc[:, xs, :], rr[0:96])
                        gcb = pu1.tile([96, 4, C_OUT], BF16, tag="gc", bufs=2)
                        nc.sync.dma_start(out=gcb, in_=Gd[:, xs, :])
                        for cc in range(4):
                            psu = pu_ps1.tile([128, 4, 96], F32, tag="pu")
                            for r in range(4):
                                nc.tensor.matmul(
                                    psu[:, r, :],
                                    gcb[:, r, cc * 128:(cc + 1) * 128],
                                    wcn[:, r, :] if tail else Wc[:, x0 + r, :],
                                    start=True, stop=True)
                            uv = U[:, cc, :, x0:x0 + 4]
                            if not tail:
                                nc.vector.tensor_mul(
                                    uv, psu.rearrange("p x y -> p y x"), rrT)
                            elif cc % 2 == 0:
                                nc.vector.tensor_copy(
                                    uv, psu.rearrange("p x y -> p y x"))
                            else:
                                nc.scalar.activation(
                                    uv, psu.rearrange("p x y -> p y x"), AF.Copy)

                    LA = 2
                    NB = W // 4
                    for i in range(NB + LA):
                        if i < NB:
                            emit_g(4 * i)
                        if i >= LA:
                            emit_agg(4 * (i - LA), tail=(i >= NB))

                # ---- phase 3b/4: row pass (denorm fused) + inc conv ----
                Uflat = U.rearrange("p a h w -> p a (h w)")
                with tc.tile_pool(name="pu2", bufs=4) as pu2, \
                     tc.tile_pool(name="pu_ps2", bufs=3, space="PSUM") as pu_ps2, \
                     tc.tile_pool(name="pd_ps2", bufs=2, space="PSUM") as pd_ps2, \
                     tc.tile_pool(name="pi", bufs=1) as pi, \
                     tc.tile_pool(name="pix", bufs=3) as pix, \
                     tc.tile_pool(name="po_ps", bufs=3, space="PSUM") as po_ps:
                    incw = pi.tile([128, KC, C_IN], BF16)
                    combb = pi.tile([128, KC], F32)
                    nc.sync.dma_start(out=incw, in_=incw_d[:])
                    nc.sync.dma_start(out=combb, in_=combb_d[:])
                    Xp = Xbf.rearrange("p a h w -> p a (h w)")

                    def emit_inc_block(b):
                        sl = slice(b * 512, (b + 1) * 512)
                        for c2 in range(KC):
                            ps = po_ps.tile([128, 512], F32, tag="po")
                            for k in range(KC):
                                nc.tensor.matmul(ps, incw[:, k, c2 * 128:(c2 + 1) * 128],
                                                 Uflat[:, k, sl],
                                                 start=(k == 0), stop=(c2 % 2 == 0 and k == KC - 1))
                            ot = pix.tile([128, 512], BF16, tag="ot")
                            if c2 % 2 == 0:
                                nc.vector.scalar_tensor_tensor(
                                    ot, ps, combb[:, c2:c2 + 1], Xp[:, c2, sl],
                                    mybir.AluOpType.add, mybir.AluOpType.add)
                            else:
                                nc.tensor.matmul(ps, iden, Xp[:, c2, sl],
                                                 start=False, stop=True)
                                nc.scalar.activation(ot, ps, AF.Identity,
                                                     bias=combb[:, c2:c2 + 1])
                            nc.sync.dma_start(out=out_d[c2][:, sl], in_=ot)

                    def emit_denorm(y0):
                        ys = slice(y0, y0 + 4)
                        sl4 = slice(y0 * 96, (y0 + 4) * 96)
                        psd2 = pd_ps2.tile([128, 4, 96], F32, tag="pd2")
                        nc.tensor.matmul(psd2, ones96b, Wr[:, ys, :],
                                         start=True, stop=False)
                        nc.tensor.matmul(psd2, ones96b, WcT[:, ys, :],
                                         start=False, stop=True)
                        rw = pu2.tile([128, 4, 96], F32, tag="rr", bufs=3)
                        nc.vector.reciprocal_approx_fast(rw, psd2)
                        # normalized copy (not in place: every col-pass psd
                        # reads all raw Wr rows - in-place would WAR-serialize
                        # the row pass behind the last agg block)
                        wrn = pu2.tile([96, 4, 96], BF16, tag="wrn", bufs=4)
                        nc.gpsimd.tensor_mul(wrn, Wr[:, ys, :], rw[0:96])
                        return wrn

                    def emit_row(y0, wrn):
                        rgb = pu2.tile([96, 4, C_OUT], BF16, tag="gt", bufs=2)
                        nc.sync.dma_start(
                            out=rgb,
                            in_=Gd[y0:y0 + 4].rearrange("y x c -> x y c"))
                        for cc in range(4):
                            psu = pu_ps2.tile([128, 4 * 96], F32, tag="pu")
                            for r in range(4):
                                nc.tensor.matmul(
                                    psu[:, r * 96:(r + 1) * 96],
                                    rgb[:, r, cc * 128:(cc + 1) * 128],
                                    wrn[:, r, :], start=True, stop=True)
                            uv = U[:, cc, y0:y0 + 4, :]
                            nc.vector.tensor_add(
                                uv, uv, psu.rearrange("p (a b) -> p a b", a=4))

                    # denorms run DLA blocks ahead of the aggregation so the
                    # normalized rows are ready when the psu matmuls arrive
                    next_b = 0
                    DLA = 2
                    wrns = {}
                    for j in range(H // 4 + DLA):
                        if j < H // 4:
                            wrns[j] = emit_denorm(4 * j)
                        if j >= DLA:
                            y0 = 4 * (j - DLA)
                            emit_row(y0, wrns.pop(j - DLA))
                            # emit inc blocks whose rows are aggregated
                            while (next_b + 1) * 512 <= y0 * 96:
                                emit_inc_block(next_b)
                                next_b += 1
                    while next_b < HW // 512:
                        emit_inc_block(next_b)
                        next_b += 1

    nc.finalize()
    return nc


def _prep_shared(t_w, t_b, f_w, f_b, g_w, g_b, inc_w, inc_b):
    bf = ml_dtypes.bfloat16
    tf_wT = np.concatenate([t_w.T, f_w.T], axis=1)  # (512, 128)
    d = {
        "tf_wT": np.ascontiguousarray(
            tf_wT.reshape(KC, 128, 128).transpose(1, 0, 2)).astype(bf),
        "g_wT": np.ascontiguousarray(
            g_w.T.reshape(KC, 128, C_OUT).transpose(1, 0, 2)).astype(bf),
        "inc_wT": np.ascontiguousarray(
            inc_w.T.reshape(KC, 128, C_IN).transpose(1, 0, 2)).astype(bf),
        "tf_b": np.concatenate([t_b, f_b]).reshape(128, 1).astype(np.float32),
        "comb_b": np.ascontiguousarray(
            (inc_b + inc_w @ g_b).reshape(KC, 128).T).astype(np.float32),
        "mask": (1.0 - np.eye(96)).astype(bf),
        "ones96b": np.ones((96, 128), bf),
        "iden128": np.eye(128, dtype=np.float32).astype(bf),
    }
    return d


def kernel(x, t_w, t_b, f_w, f_b, g_w, g_b, inc_w, inc_b):
    x = np.asarray(x, dtype=np.float32)
    shared = _prep_shared(
        np.asarray(t_w, np.float32), np.asarray(t_b, np.float32),
        np.asarray(f_w, np.float32), np.asarray(f_b, np.float32),
        np.asarray(g_w, np.float32), np.asarray(g_b, np.float32),
        np.asarray(inc_w, np.float32), np.asarray(inc_b, np.float32))

    bf = ml_dtypes.bfloat16
    in_maps = []
    for n in range(N):
        xi = x[n].reshape(KC, 128, HW)  # (4, 128, 9216)
        m = dict(shared)
        m["x_bf"] = np.ascontiguousarray(xi.transpose(1, 0, 2)).astype(bf)
        in_maps.append(m)

    if "nc" not in _cache:
        _cache["nc"] = build_program()
    res = run_bass_kernel_spmd(_cache["nc"], in_maps, core_ids=list(range(N)))
    out = np.stack([r["out"].reshape(C_IN, H, W) for r in res.results])
    return out.astype(np.float32)


if __name__ == "__main__":
    rng = np.random.default_rng(0)
    ins = {
        "x": rng.standard_normal((N, C_IN, H, W), dtype=np.float32),
        "t_w": rng.standard_normal((C_INNER, C_IN), dtype=np.float32) * 0.02,
        "t_b": np.zeros(C_INNER, np.float32),
        "f_w": rng.standard_normal((C_INNER, C_IN), dtype=np.float32) * 0.02,
        "f_b": np.zeros(C_INNER, np.float32),
        "g_w": rng.standard_normal((C_OUT, C_IN), dtype=np.float32) * 0.02,
        "g_b": np.zeros(C_OUT, np.float32),
        "inc_w": rng.standard_normal((C_IN, C_OUT), dtype=np.float32) * 0.02,
        "inc_b": np.zeros(C_IN, np.float32),
    }
    y = kernel(**ins)
    print(y.shape, y.dtype)



# revision 4
# speedup vs baseline: 1.2710x; 1.2710x over previous
"""Criss-cross attention (CCNet) kernel for 8 TRN2 NeuronCores.

Data-parallel over batch N=8: one image per core.

v3 restructure (vs v2):
  - inc conv eliminated algebraically: conv1x1(ca_map(a, g), inc_w) ==
    ca_map(a, conv1x1(x, inc_w@g_w)) since channel mixing commutes with the
    per-channel pixel-weighted sums. Device computes h = M x (M = inc_w@g_w,
    host-fused) and aggregates h directly; host adds x + comb_b in f32.
  - fp8 e4m3 everywhere heavy: x, tf weights, M weights shipped fp8; both
    convs run MatmulPerfMode.DoubleRow (2 contraction chunks per matmul at
    2x rate); h stored fp8 in DRAM (Gd); aggregation matmuls fp8.
  - h conv runs on flat 128-pixel blocks (full partition use) writing
    contiguous Gd rows; col/row aggregation reads Gd slabs (DMA does the
    pixel transpose for the row pass).
  - softmax denominators via (1/S2)-matmul broadcast; fast reciprocal on
    DVE; normalized weights pre-scaled to fp8 on Pool, so aggregation
    drains are single scaled copies (Act) / scaled-add STT (DVE).
"""

import sys

sys.path.insert(0, "/opt/trn_rl_repo")

from contextlib import ExitStack

import numpy as np
import ml_dtypes

import concourse.bass as bass
import concourse.bacc as bacc
import concourse.tile as tile
from concourse import mybir
from concourse.bass_utils import run_bass_kernel_spmd

BF16 = mybir.dt.bfloat16
F32 = mybir.dt.float32
FP8 = mybir.dt.float8e4
AF = mybir.ActivationFunctionType
DR = mybir.MatmulPerfMode.DoubleRow

N, C_IN, C_INNER, C_OUT, H, W = 8, 512, 64, 512, 96, 96
HW = H * W  # 9216
KC = C_IN // 128  # 4 contraction chunks

S0 = 16.0     # x scale (fp8)
STF = 256.0   # t/f weight scale
SM = 2048.0   # M weight scale
S1 = 64.0     # Gd (h) storage scale
S2 = 128.0    # normalized-weight scale; folded into the denominator "ones"

_cache = {}


def build_program():
    nc = bacc.Bacc()

    # ---- DRAM I/O ----
    xq_d = nc.dram_tensor("x_q", (128, KC, HW), FP8, kind="ExternalInput")
    tfw_d = nc.dram_tensor("tf_wT", (128, KC, 128), FP8, kind="ExternalInput")
    mw_d = nc.dram_tensor("m_wT", (128, KC, C_OUT), FP8, kind="ExternalInput")
    tfb_d = nc.dram_tensor("tf_b", (128, 1), F32, kind="ExternalInput")
    mask_d = nc.dram_tensor("mask", (96, 96), BF16, kind="ExternalInput")
    ones_d = nc.dram_tensor("ones_s", (96, 128), BF16, kind="ExternalInput")
    out_d = nc.dram_tensor("out", (KC, 128, HW), BF16, kind="ExternalOutput")

    with ExitStack() as ctx:
        tc = ctx.enter_context(tile.TileContext(nc))
        p0 = ctx.enter_context(tc.tile_pool(name="p0", bufs=1))

        # ---- persistent tiles ----
        Xq = p0.tile([128, KC, H, W], FP8)
        ones_s = p0.tile([96, 128], BF16)  # value 1/S2
        mask = p0.tile([96, 96], BF16)
        mw = p0.tile([128, KC, C_OUT], FP8)

        nc.sync.dma_start(out=ones_s, in_=ones_d[:])
        nc.sync.dma_start(out=mask, in_=mask_d[:])
        nc.sync.dma_start(out=mw, in_=mw_d[:])
        xv = xq_d[:].rearrange("p a (h w) -> p a h w", h=H)

        # TF (phases 1-2) and U (phases 4-5) share one big slot.
        TF = p0.tile([128, 2, H, W], BF16, tag="big", name="TF")
        T = TF[0:64, 0]
        F = TF[0:64, 1]

        with tc.tile_pool(name="pwr", bufs=1) as pwr:
            Wr = pwr.tile([96, H, W], BF16)   # exp(row affinity)[i, y, x]
            with tc.tile_pool(name="pwc", bufs=1) as pwc:
                Wc = pwc.tile([96, W, H], BF16)   # exp(col affinity)[j, x, y]
                wcn = pwc.tile([96, W, H], FP8)   # Wc * S2/D
                wrn = pwr.tile([96, H, W], FP8)   # Wr * S2/D

                # ---- phase 1: t/f conv (fp8 DoubleRow) ----
                with tc.tile_pool(name="pe", bufs=1) as pe, \
                     tc.tile_pool(name="pe_ps", bufs=3, space="PSUM") as pe_ps, \
                     tc.tile_pool(name="ptf_ps", bufs=2, space="PSUM") as ptf_ps:
                    tfw = pe.tile([128, KC, 128], FP8)
                    tfb = pe.tile([128, 1], F32)
                    nc.sync.dma_start(out=tfw, in_=tfw_d[:])
                    nc.sync.dma_start(out=tfb, in_=tfb_d[:])
                    for q in range(8):
                        nc.sync.dma_start(
                            out=Xq[:, :, q * 12:(q + 1) * 12, :],
                            in_=xv[:, :, q * 12:(q + 1) * 12, :])

                    Xflat = Xq.rearrange("p a h w -> p a (h w)")
                    TFflat = TF.rearrange("p c h w -> p c (h w)")
                    for b in range(HW // 512):
                        sl = slice(b * 512, (b + 1) * 512)
                        pst = ptf_ps.tile([128, 512], F32, tag="pt")
                        for t in range(KC // 2):
                            nc.tensor.matmul(
                                pst, tfw[:, 2 * t:2 * t + 2, :],
                                Xflat[:, 2 * t:2 * t + 2, sl],
                                start=(t == 0), stop=(t == KC // 2 - 1),
                                perf_mode=DR)
                        nc.scalar.activation(TFflat[:, 0, sl], pst,
                                             AF.Identity, bias=tfb,
                                             scale=1.0 / (S0 * STF))
                        # re-base f to partitions 0:64 (plane 1) for matmul
                        if b % 3 == 2:
                            sl3 = slice((b - 2) * 512, (b + 1) * 512)
                            nc.sync.dma_start(out=TFflat[0:64, 1, sl3],
                                              in_=TFflat[64:128, 0, sl3])

                    # ---- phase 2: affinities + exp ----
                    # row: E[i, x] = sum_c f[c,y,i] t[c,y,x]
                    for y0 in range(0, H, 8):
                        ps = pe_ps.tile([96, 2, 512], F32, tag="pe")
                        for r in range(8):
                            nc.tensor.matmul(
                                ps[:, r // 4, (r % 4) * 96:(r % 4) * 96 + 96],
                                F[:, y0 + r, :], T[:, y0 + r, :],
                                start=True, stop=True)
                        nc.scalar.activation(
                            Wr[:, y0:y0 + 8, :].rearrange(
                                "i (a b) w -> i a (b w)", a=2),
                            ps[:, :, 0:384], AF.Exp)
                    # col: E[j, y] = sum_c f[c,j,x] t[c,y,x]; kill j==y (DVE)
                    mb8 = bass.AP(tensor=mask.tensor, offset=mask.offset,
                                  ap=[mask.ap[0], [0, 8], mask.ap[1]])
                    for x0 in range(0, W, 8):
                        ps = pe_ps.tile([96, 2, 512], F32, tag="pe")
                        for r in range(8):
                            nc.tensor.matmul(
                                ps[:, r // 4, (r % 4) * 96:(r % 4) * 96 + 96],
                                F[:, :, x0 + r], T[:, :, x0 + r],
                                start=True, stop=True)
                        wcs = Wc[:, x0:x0 + 8, :]
                        nc.scalar.activation(
                            wcs.rearrange("j (a b) y -> j a (b y)", a=2),
                            ps[:, :, 0:384], AF.Exp)
                        nc.vector.tensor_mul(wcs, wcs, mb8)

                WrT = Wr.rearrange("i h w -> i w h")
                WcT = Wc.rearrange("j x y -> j y x")

                # ---- phase 2.5: denominators -> fp8 normalized weights ----
                # psd = (1/S2) * (sum_j Wc + sum_i Wr), broadcast over 128
                # partitions by the matmul; rr = S2/D on DVE; Pool writes
                # wcn/wrn = W * S2/D as fp8.
                pgd = ctx.enter_context(tc.tile_pool(name="pgd", bufs=1,
                                                     space="DRAM"))
                Gd = pgd.tile([H, W, C_OUT], FP8)  # h * S1, [y, x, c] flat
                Gflat = Gd.rearrange("h w c -> (h w) c")

                with tc.tile_pool(name="pn", bufs=6) as pn, \
                     tc.tile_pool(name="pd_ps", bufs=3, space="PSUM") as pd_ps, \
                     tc.tile_pool(name="pg_ps", bufs=3, space="PSUM") as pg_ps, \
                     tc.tile_pool(name="pgt", bufs=3) as pgt:
                    for xb in range(W // 4):
                        xs = slice(xb * 4, xb * 4 + 4)
                        psd = pd_ps.tile([128, 4, 96], F32, tag="pd")
                        nc.tensor.matmul(psd, ones_s, Wc[:, xs, :],
                                         start=True, stop=False)
                        nc.tensor.matmul(psd, ones_s, WrT[:, xs, :],
                                         start=False, stop=True)
                        rr = pn.tile([128, 4, 96], F32, tag="rr")
                        nc.vector.reciprocal_approx_fast(rr, psd)
                        nc.gpsimd.tensor_mul(wcn[:, xs, :], Wc[:, xs, :],
                                             rr[0:96])
                    for yb in range(H // 4):
                        ys = slice(yb * 4, yb * 4 + 4)
                        psd = pd_ps.tile([128, 4, 96], F32, tag="pd")
                        nc.tensor.matmul(psd, ones_s, Wr[:, ys, :],
                                         start=True, stop=False)
                        nc.tensor.matmul(psd, ones_s, WcT[:, ys, :],
                                         start=False, stop=True)
                        rr = pn.tile([128, 4, 96], F32, tag="rr")
                        nc.vector.reciprocal_approx_fast(rr, psd)
                        nc.gpsimd.tensor_mul(wrn[:, ys, :], Wr[:, ys, :],
                                             rr[0:96])

                    # ---- phase 3: h conv (fp8 DR, flat 128-pixel blocks) ----
                    # Drains split Act/Pool; Gd written 2 blocks per DMA on
                    # the Act HWDGE queue.
                    hsc = S1 / (S0 * SM)
                    for bp in range(HW // 256):
                        gtb = pgt.tile([128, 2, C_OUT], FP8, tag="gt")
                        for half in range(2):
                            b = 2 * bp + half
                            psg = pg_ps.tile([128, C_OUT], F32, tag="pg")
                            for t in range(KC // 2):
                                nc.tensor.matmul(
                                    psg,
                                    Xflat[:, 2 * t:2 * t + 2,
                                          b * 128:(b + 1) * 128],
                                    mw[:, 2 * t:2 * t + 2, :],
                                    start=(t == 0), stop=(t == KC // 2 - 1),
                                    perf_mode=DR)
                            nc.scalar.activation(gtb[:, half, :], psg,
                                                 AF.Copy, scale=hsc)
                        gdv = Gflat[bp * 256:(bp + 1) * 256].rearrange(
                            "(a p) c -> p a c", a=2)
                        nc.scalar.dma_start(out=gdv, in_=gtb)

                # ---- phase 4: col pass (writes U = out, bf16) ----
                U = p0.tile([128, KC, H, W], BF16, tag="big", name="U")
                usc = 1.0 / (S1 * S2)
                with tc.tile_pool(name="pu1", bufs=3) as pu1, \
                     tc.tile_pool(name="pu_ps1", bufs=4, space="PSUM") as pu_ps1:
                    for xb in range(W // 4):
                        x0 = xb * 4
                        xs = slice(x0, x0 + 4)
                        gcb = pu1.tile([96, 4, C_OUT], FP8, tag="gc")
                        nc.sync.dma_start(out=gcb, in_=Gd[:, xs, :])
                        for cc in range(4):
                            psu = pu_ps1.tile([128, 4, 96], F32, tag="pu")
                            for r in range(4):
                                nc.tensor.matmul(
                                    psu[:, r, :],
                                    gcb[:, r, cc * 128:(cc + 1) * 128],
                                    wcn[:, x0 + r, :],
                                    start=True, stop=True)
                            nc.scalar.activation(
                                U[:, cc, :, xs],
                                psu.rearrange("p x y -> p y x"),
                                AF.Copy, scale=usc)

                # ---- phase 5: row pass (U += row agg) + out DMA ----
                outv = out_d[:].rearrange("k p q -> p k q")
                with tc.tile_pool(name="pu2", bufs=3) as pu2, \
                     tc.tile_pool(name="pu_ps2", bufs=4, space="PSUM") as pu_ps2:
                    for yb in range(H // 4):
                        y0 = yb * 4
                        ys = slice(y0, y0 + 4)
                        rgb = pu2.tile([96, 4, C_OUT], FP8, tag="gt")
                        nc.sync.dma_start(
                            out=rgb,
                            in_=Gd[ys].rearrange("y x c -> x y c"))
                        for cc in range(4):
                            psu = pu_ps2.tile([128, 4 * 96], F32, tag="pu")
                            for r in range(4):
                                nc.tensor.matmul(
                                    psu[:, r * 96:(r + 1) * 96],
                                    rgb[:, r, cc * 128:(cc + 1) * 128],
                                    wrn[:, y0 + r, :], start=True, stop=True)
                            uv = U[:, cc, ys, :]
                            nc.vector.scalar_tensor_tensor(
                                uv, psu.rearrange("p (a b) -> p a b", a=4),
                                usc, uv,
                                mybir.AluOpType.mult, mybir.AluOpType.add)
                        nc.scalar.dma_start(
                            out=outv[:, :, y0 * 96:(y0 + 4) * 96],
                            in_=U[:, :, ys, :])

    nc.finalize()
    return nc


def _prep_shared(t_w, t_b, f_w, f_b, g_w, g_b, inc_w, inc_b):
    bf = ml_dtypes.bfloat16
    f8 = ml_dtypes.float8_e4m3
    tf_wT = np.concatenate([t_w.T, f_w.T], axis=1)  # (512, 128)
    M = inc_w @ g_w  # (512, 512)
    d = {
        "tf_wT": np.ascontiguousarray(
            (tf_wT * STF).reshape(KC, 128, 128).transpose(1, 0, 2)).astype(f8),
        "m_wT": np.ascontiguousarray(
            (M.T * SM).reshape(KC, 128, C_OUT).transpose(1, 0, 2)).astype(f8),
        "tf_b": np.concatenate([t_b, f_b]).reshape(128, 1).astype(np.float32),
        "mask": (1.0 - np.eye(96)).astype(bf),
        "ones_s": np.full((96, 128), 1.0 / S2, dtype=np.float32).astype(bf),
    }
    comb_b = inc_b + inc_w @ g_b
    return d, comb_b


def kernel(x, t_w, t_b, f_w, f_b, g_w, g_b, inc_w, inc_b):
    x = np.asarray(x, dtype=np.float32)
    shared, comb_b = _prep_shared(
        np.asarray(t_w, np.float32), np.asarray(t_b, np.float32),
        np.asarray(f_w, np.float32), np.asarray(f_b, np.float32),
        np.asarray(g_w, np.float32), np.asarray(g_b, np.float32),
        np.asarray(inc_w, np.float32), np.asarray(inc_b, np.float32))

    f8 = ml_dtypes.float8_e4m3
    in_maps = []
    for n in range(N):
        xi = x[n].reshape(KC, 128, HW)  # (4, 128, 9216)
        m = dict(shared)
        m["x_q"] = np.ascontiguousarray(
            xi.transpose(1, 0, 2) * S0).astype(f8)
        in_maps.append(m)

    if "nc" not in _cache:
        _cache["nc"] = build_program()
    res = run_bass_kernel_spmd(_cache["nc"], in_maps, core_ids=list(range(N)))
    attn = np.stack([r["out"].astype(np.float32).reshape(C_IN, H, W)
                     for r in res.results])
    return x + attn + comb_b.astype(np.float32)[None, :, None, None]


if __name__ == "__main__":
    rng = np.random.default_rng(0)
    ins = {
        "x": rng.standard_normal((N, C_IN, H, W), dtype=np.float32),
        "t_w": rng.standard_normal((C_INNER, C_IN), dtype=np.float32) * 0.02,
        "t_b": np.zeros(C_INNER, np.float32),
        "f_w": rng.standard_normal((C_INNER, C_IN), dtype=np.float32) * 0.02,
        "f_b": np.zeros(C_INNER, np.float32),
        "g_w": rng.standard_normal((C_OUT, C_IN), dtype=np.float32) * 0.02,
        "g_b": np.zeros(C_OUT, np.float32),
        "inc_w": rng.standard_normal((C_IN, C_OUT), dtype=np.float32) * 0.02,
        "inc_b": np.zeros(C_IN, np.float32),
    }
    y = kernel(**ins)
    print(y.shape, y.dtype)


# revision 10
# speedup vs baseline: 1.3082x; 1.0293x over previous
"""Criss-cross attention (CCNet) kernel for 8 TRN2 NeuronCores.

Data-parallel over batch N=8: one image per core.

v3 restructure (vs v2):
  - inc conv eliminated algebraically: conv1x1(ca_map(a, g), inc_w) ==
    ca_map(a, conv1x1(x, inc_w@g_w)) since channel mixing commutes with the
    per-channel pixel-weighted sums. Device computes h = M x (M = inc_w@g_w,
    host-fused) and aggregates h directly; host adds x + comb_b in f32.
  - fp8 e4m3 everywhere heavy: x, tf weights, M weights shipped fp8; both
    convs run MatmulPerfMode.DoubleRow (2 contraction chunks per matmul at
    2x rate); h stored fp8 in DRAM (Gd); aggregation matmuls fp8.
  - h conv runs on flat 128-pixel blocks (full partition use) writing
    contiguous Gd rows; col/row aggregation reads Gd slabs (DMA does the
    pixel transpose for the row pass).
  - softmax denominators via (1/S2)-matmul broadcast; fast reciprocal on
    DVE; normalized weights pre-scaled to fp8 on Pool, so aggregation
    drains are single scaled copies (Act) / scaled-add STT (DVE).
"""

import sys

sys.path.insert(0, "/opt/trn_rl_repo")

from contextlib import ExitStack

import numpy as np
import ml_dtypes

import concourse.bass as bass
import concourse.bacc as bacc
import concourse.tile as tile
from concourse import mybir
from concourse.bass_utils import run_bass_kernel_spmd

BF16 = mybir.dt.bfloat16
F32 = mybir.dt.float32
FP8 = mybir.dt.float8e4
AF = mybir.ActivationFunctionType
DR = mybir.MatmulPerfMode.DoubleRow

N, C_IN, C_INNER, C_OUT, H, W = 8, 512, 64, 512, 96, 96
HW = H * W  # 9216
KC = C_IN // 128  # 4 contraction chunks

S0 = 16.0     # x scale (fp8)
STF = 256.0   # t/f weight scale
SM = 2048.0   # M weight scale
S1 = 64.0     # Gd (h) storage scale
S2 = 128.0    # normalized-weight scale; folded into the denominator "ones"

_cache = {}


def build_program():
    nc = bacc.Bacc()

    # ---- DRAM I/O ----
    xq_d = nc.dram_tensor("x_q", (128, KC, HW), FP8, kind="ExternalInput")
    tfw_d = nc.dram_tensor("tf_wT", (128, KC, 128), FP8, kind="ExternalInput")
    mw_d = nc.dram_tensor("m_wT", (128, KC, C_OUT), FP8, kind="ExternalInput")
    tfb_d = nc.dram_tensor("tf_b", (128, 1), F32, kind="ExternalInput")
    mask_d = nc.dram_tensor("mask", (96, 96), BF16, kind="ExternalInput")
    ones_d = nc.dram_tensor("ones_s", (96, 128), BF16, kind="ExternalInput")
    out_d = nc.dram_tensor("out", (KC, 128, HW), BF16, kind="ExternalOutput")

    with ExitStack() as ctx:
        tc = ctx.enter_context(tile.TileContext(nc))
        p0 = ctx.enter_context(tc.tile_pool(name="p0", bufs=1))

        # ---- persistent tiles ----
        Xq = p0.tile([128, KC, H, W], FP8)
        ones_s = p0.tile([96, 128], BF16)  # value 1/S2
        mask = p0.tile([96, 96], BF16)
        mw = p0.tile([128, KC, C_OUT], FP8)

        nc.sync.dma_start(out=ones_s, in_=ones_d[:])
        nc.sync.dma_start(out=mask, in_=mask_d[:])
        nc.sync.dma_start(out=mw, in_=mw_d[:])
        xv = xq_d[:].rearrange("p a (h w) -> p a h w", h=H)

        # TF (phases 1-2) and U (phases 4-5) share one big slot.
        TF = p0.tile([128, 2, H, W], BF16, tag="big", name="TF")
        T = TF[0:64, 0]
        F = TF[0:64, 1]

        with tc.tile_pool(name="pwr", bufs=1) as pwr:
            Wr = pwr.tile([96, H, W], BF16)   # exp(row affinity)[i, y, x]
            with tc.tile_pool(name="pwc", bufs=1) as pwc:
                Wc = pwc.tile([96, W, H], BF16)   # exp(col affinity)[j, x, y]
                wcn = pwc.tile([96, W, H], FP8)   # Wc * S2/D
                wrn = pwr.tile([96, H, W], FP8)   # Wr * S2/D

                # ---- phase 1: t/f conv (fp8 DoubleRow) ----
                with tc.tile_pool(name="pe", bufs=1) as pe, \
                     tc.tile_pool(name="pe_ps", bufs=3, space="PSUM") as pe_ps, \
                     tc.tile_pool(name="ptf_ps", bufs=2, space="PSUM") as ptf_ps:
                    tfw = pe.tile([128, KC, 128], FP8)
                    tfb = pe.tile([128, 1], F32)
                    nc.sync.dma_start(out=tfw, in_=tfw_d[:])
                    nc.sync.dma_start(out=tfb, in_=tfb_d[:])
                    tfbb = bass.AP(tensor=tfb.tensor, offset=tfb.offset,
                                   ap=[tfb.ap[0], [0, 512]])
                    for q in range(8):
                        nc.sync.dma_start(
                            out=Xq[:, :, q * 12:(q + 1) * 12, :],
                            in_=xv[:, :, q * 12:(q + 1) * 12, :])

                    Xflat = Xq.rearrange("p a h w -> p a (h w)")
                    TFflat = TF.rearrange("p c h w -> p c (h w)")
                    for b in range(HW // 512):
                        sl = slice(b * 512, (b + 1) * 512)
                        pst = ptf_ps.tile([128, 512], F32, tag="pt")
                        for t in range(KC // 2):
                            nc.tensor.matmul(
                                pst, tfw[:, 2 * t:2 * t + 2, :],
                                Xflat[:, 2 * t:2 * t + 2, sl],
                                start=(t == 0), stop=(t == KC // 2 - 1),
                                perf_mode=DR)
                        if b % 2 == 0:
                            nc.scalar.activation(TFflat[:, 0, sl], pst,
                                                 AF.Identity, bias=tfb,
                                                 scale=1.0 / (S0 * STF))
                        else:
                            nc.vector.scalar_tensor_tensor(
                                TFflat[:, 0, sl], pst, 1.0 / (S0 * STF), tfbb,
                                mybir.AluOpType.mult, mybir.AluOpType.add)
                        # re-base f to partitions 0:64 (plane 1) for matmul
                        if b % 3 == 2:
                            sl3 = slice((b - 2) * 512, (b + 1) * 512)
                            nc.sync.dma_start(out=TFflat[0:64, 1, sl3],
                                              in_=TFflat[64:128, 0, sl3])

                    # ---- phase 2: affinities + exp ----
                    # row: E[i, x] = sum_c f[c,y,i] t[c,y,x]
                    for y0 in range(0, H, 8):
                        ps = pe_ps.tile([96, 2, 512], F32, tag="pe")
                        for r in range(8):
                            nc.tensor.matmul(
                                ps[:, r // 4, (r % 4) * 96:(r % 4) * 96 + 96],
                                F[:, y0 + r, :], T[:, y0 + r, :],
                                start=True, stop=True)
                        nc.scalar.activation(
                            Wr[:, y0:y0 + 8, :].rearrange(
                                "i (a b) w -> i a (b w)", a=2),
                            ps[:, :, 0:384], AF.Exp)
                    # col: E[j, y] = sum_c f[c,j,x] t[c,y,x]; kill j==y (DVE)
                    mb8 = bass.AP(tensor=mask.tensor, offset=mask.offset,
                                  ap=[mask.ap[0], [0, 8], mask.ap[1]])
                    for x0 in range(0, W, 8):
                        ps = pe_ps.tile([96, 2, 512], F32, tag="pe")
                        for r in range(8):
                            nc.tensor.matmul(
                                ps[:, r // 4, (r % 4) * 96:(r % 4) * 96 + 96],
                                F[:, :, x0 + r], T[:, :, x0 + r],
                                start=True, stop=True)
                        wcs = Wc[:, x0:x0 + 8, :]
                        nc.scalar.activation(
                            wcs.rearrange("j (a b) y -> j a (b y)", a=2),
                            ps[:, :, 0:384], AF.Exp)
                        nc.vector.tensor_mul(wcs, wcs, mb8)

                WrT = Wr.rearrange("i h w -> i w h")
                WcT = Wc.rearrange("j x y -> j y x")

                # ---- phase 2.5: denominators -> fp8 normalized weights ----
                # psd = (1/S2) * (sum_j Wc + sum_i Wr), broadcast over 128
                # partitions by the matmul; rr = S2/D on DVE; Pool writes
                # wcn/wrn = W * S2/D as fp8.
                pgd = ctx.enter_context(tc.tile_pool(name="pgd", bufs=1,
                                                     space="DRAM"))
                Gd = pgd.tile([H, W, C_OUT], FP8)  # h * S1, [y, x, c] flat
                Gflat = Gd.rearrange("h w c -> (h w) c")

                with tc.tile_pool(name="pn", bufs=6) as pn, \
                     tc.tile_pool(name="pd_ps", bufs=3, space="PSUM") as pd_ps, \
                     tc.tile_pool(name="pg_ps", bufs=3, space="PSUM") as pg_ps, \
                     tc.tile_pool(name="pgt", bufs=3) as pgt:
                    for xb in range(W // 4):
                        xs = slice(xb * 4, xb * 4 + 4)
                        psd = pd_ps.tile([128, 4, 96], F32, tag="pd")
                        nc.tensor.matmul(psd, ones_s, Wc[:, xs, :],
                                         start=True, stop=False)
                        nc.tensor.matmul(psd, ones_s, WrT[:, xs, :],
                                         start=False, stop=True)
                        rr = pn.tile([128, 4, 96], F32, tag="rr")
                        nc.vector.reciprocal_approx_fast(rr, psd)
                        nc.gpsimd.tensor_mul(wcn[:, xs, :], Wc[:, xs, :],
                                             rr[0:96])
                    for yb in range(H // 4):
                        ys = slice(yb * 4, yb * 4 + 4)
                        psd = pd_ps.tile([128, 4, 96], F32, tag="pd")
                        nc.tensor.matmul(psd, ones_s, Wr[:, ys, :],
                                         start=True, stop=False)
                        nc.tensor.matmul(psd, ones_s, WcT[:, ys, :],
                                         start=False, stop=True)
                        rr = pn.tile([128, 4, 96], F32, tag="rr")
                        nc.vector.reciprocal_approx_fast(rr, psd)
                        nc.gpsimd.tensor_mul(wrn[:, ys, :], Wr[:, ys, :],
                                             rr[0:96])

                    # ---- phase 3: h conv (fp8 DR, flat 128-pixel blocks) ----
                    # Drains split Act/Pool; Gd written 2 blocks per DMA on
                    # the Act HWDGE queue.
                    hsc = S1 / (S0 * SM)
                    for bp in range(HW // 256):
                        gtb = pgt.tile([128, 2, C_OUT], FP8, tag="gt")
                        for half in range(2):
                            b = 2 * bp + half
                            psg = pg_ps.tile([128, C_OUT], F32, tag="pg")
                            for t in range(KC // 2):
                                nc.tensor.matmul(
                                    psg,
                                    Xflat[:, 2 * t:2 * t + 2,
                                          b * 128:(b + 1) * 128],
                                    mw[:, 2 * t:2 * t + 2, :],
                                    start=(t == 0), stop=(t == KC // 2 - 1),
                                    perf_mode=DR)
                            if half == 0:
                                nc.scalar.activation(gtb[:, half, :], psg,
                                                     AF.Copy, scale=hsc)
                            else:
                                nc.vector.tensor_scalar_mul(
                                    gtb[:, half, :], psg, hsc)
                        gdv = Gflat[bp * 256:(bp + 1) * 256].rearrange(
                            "(a p) c -> p a c", a=2)
                        nc.scalar.dma_start(out=gdv, in_=gtb)

                # ---- phase 4: col pass (writes U = out, bf16) ----
                U = p0.tile([128, KC, H, W], BF16, tag="big", name="U")
                usc = 1.0 / (S1 * S2)
                with tc.tile_pool(name="pu1", bufs=3) as pu1, \
                     tc.tile_pool(name="pu_ps1", bufs=3, space="PSUM") as pu_ps1:
                    for xb in range(W // 4):
                        x0 = xb * 4
                        xs = slice(x0, x0 + 4)
                        gcb = pu1.tile([96, 4, C_OUT], FP8, tag="gc")
                        nc.sync.dma_start(out=gcb, in_=Gd[:, xs, :])
                        for ch in range(2):
                            psu = pu_ps1.tile([128, 2, 512], F32, tag="pu")
                            for c2 in range(2):
                                cc = 2 * ch + c2
                                for r in range(4):
                                    nc.tensor.matmul(
                                        psu[:, c2, r * 96:(r + 1) * 96],
                                        gcb[:, r, cc * 128:(cc + 1) * 128],
                                        wcn[:, x0 + r, :],
                                        start=True, stop=True)
                            uv = U[:, 2 * ch:2 * ch + 2, :, xs]
                            psv = psu[:, :, 0:384].rearrange(
                                "p c (x y) -> p c y x", x=4)
                            if (2 * xb + ch) % 2 == 0:
                                nc.scalar.activation(uv, psv, AF.Copy,
                                                     scale=usc)
                            else:
                                nc.vector.tensor_scalar_mul(uv, psv, usc)

                # ---- phase 5: row pass (U += row agg) + out DMA ----
                outv = out_d[:].rearrange("k p q -> p k q")
                with tc.tile_pool(name="pu2", bufs=3) as pu2, \
                     tc.tile_pool(name="pu_ps2", bufs=3, space="PSUM") as pu_ps2, \
                     tc.tile_pool(name="pst2", bufs=3) as pst2:
                    for yb in range(H // 4):
                        y0 = yb * 4
                        ys = slice(y0, y0 + 4)
                        rgb = pu2.tile([96, 4, C_OUT], FP8, tag="gt")
                        nc.sync.dma_start(
                            out=rgb,
                            in_=Gd[ys].rearrange("y x c -> x y c"))
                        for ch in range(2):
                            psu = pu_ps2.tile([128, 2, 512], F32, tag="pu")
                            for c2 in range(2):
                                cc = 2 * ch + c2
                                for r in range(4):
                                    nc.tensor.matmul(
                                        psu[:, c2, r * 96:(r + 1) * 96],
                                        rgb[:, r, cc * 128:(cc + 1) * 128],
                                        wrn[:, y0 + r, :],
                                        start=True, stop=True)
                            uv = U[:, 2 * ch:2 * ch + 2, ys, :]
                            psv = psu[:, :, 0:384].rearrange(
                                "p c (a b) -> p c a b", a=4)
                            if (2 * yb + ch) % 3 == 2:
                                # Act scaled copy + Pool add (Pool is idle;
                                # Act cannot do tensor+tensor)
                                stg = pst2.tile([128, 2, 4, 96], BF16,
                                                tag="st")
                                nc.scalar.activation(stg, psv, AF.Copy,
                                                     scale=usc)
                                nc.gpsimd.tensor_add(uv, uv, stg)
                            else:
                                nc.vector.scalar_tensor_tensor(
                                    uv, psv, usc, uv,
                                    mybir.AluOpType.mult,
                                    mybir.AluOpType.add)
                        nc.scalar.dma_start(
                            out=outv[:, :, y0 * 96:(y0 + 4) * 96],
                            in_=U[:, :, ys, :])

    nc.finalize()
    return nc


def _prep_shared(t_w, t_b, f_w, f_b, g_w, g_b, inc_w, inc_b):
    bf = ml_dtypes.bfloat16
    f8 = ml_dtypes.float8_e4m3
    tf_wT = np.concatenate([t_w.T, f_w.T], axis=1)  # (512, 128)
    M = inc_w @ g_w  # (512, 512)
    d = {
        "tf_wT": np.ascontiguousarray(
            (tf_wT * STF).reshape(KC, 128, 128).transpose(1, 0, 2)).astype(f8),
        "m_wT": np.ascontiguousarray(
            (M.T * SM).reshape(KC, 128, C_OUT).transpose(1, 0, 2)).astype(f8),
        "tf_b": np.concatenate([t_b, f_b]).reshape(128, 1).astype(np.float32),
        "mask": (1.0 - np.eye(96)).astype(bf),
        "ones_s": np.full((96, 128), 1.0 / S2, dtype=np.float32).astype(bf),
    }
    comb_b = inc_b + inc_w @ g_b
    return d, comb_b


def kernel(x, t_w, t_b, f_w, f_b, g_w, g_b, inc_w, inc_b):
    x = np.asarray(x, dtype=np.float32)
    shared, comb_b = _prep_shared(
        np.asarray(t_w, np.float32), np.asarray(t_b, np.float32),
        np.asarray(f_w, np.float32), np.asarray(f_b, np.float32),
        np.asarray(g_w, np.float32), np.asarray(g_b, np.float32),
        np.asarray(inc_w, np.float32), np.asarray(inc_b, np.float32))

    f8 = ml_dtypes.float8_e4m3
    in_maps = []
    for n in range(N):
        xi = x[n].reshape(KC, 128, HW)  # (4, 128, 9216)
        m = dict(shared)
        m["x_q"] = np.ascontiguousarray(
            xi.transpose(1, 0, 2) * S0).astype(f8)
        in_maps.append(m)

    if "nc" not in _cache:
        _cache["nc"] = build_program()
    res = run_bass_kernel_spmd(_cache["nc"], in_maps, core_ids=list(range(N)))
    attn = np.stack([r["out"].astype(np.float32).reshape(C_IN, H, W)
                     for r in res.results])
    return x + attn + comb_b.astype(np.float32)[None, :, None, None]


if __name__ == "__main__":
    rng = np.random.default_rng(0)
    ins = {
        "x": rng.standard_normal((N, C_IN, H, W), dtype=np.float32),
        "t_w": rng.standard_normal((C_INNER, C_IN), dtype=np.float32) * 0.02,
        "t_b": np.zeros(C_INNER, np.float32),
        "f_w": rng.standard_normal((C_INNER, C_IN), dtype=np.float32) * 0.02,
        "f_b": np.zeros(C_INNER, np.float32),
        "g_w": rng.standard_normal((C_OUT, C_IN), dtype=np.float32) * 0.02,
        "g_b": np.zeros(C_OUT, np.float32),
        "inc_w": rng.standard_normal((C_IN, C_OUT), dtype=np.float32) * 0.02,
        "inc_b": np.zeros(C_IN, np.float32),
    }
    y = kernel(**ins)
    print(y.shape, y.dtype)


# revision 14
# speedup vs baseline: 1.9893x; 1.5206x over previous
"""Criss-cross attention (CCNet) kernel for 8 TRN2 NeuronCores.

Data-parallel over batch N=8: one image per core.

v4 (vs v3): phase-fused schedule.
  - inc conv eliminated algebraically: conv1x1(ca_map(a, g), inc_w) ==
    ca_map(a, conv1x1(x, inc_w@g_w)); host adds x + comb_b in f32.
  - fp8 e4m3 throughout: x / tf / M weights shipped fp8, convs run
    MatmulPerfMode.DoubleRow; h stored fp8 in DRAM; aggregation matmuls fp8;
    U (attn output) fp8 with x64 scale, decoded on host.
  - h conv on flat 128-pixel blocks writing contiguous Gd rows.
  - schedule: h-conv blocks interleaved into the affinity phase (conv drains
    on DVE there, exps on Act) and into the denominator phase; col+row
    aggregation share one PSUM pool/scope so PE runs ahead across passes.
  - denominators via (1/S2)-valued ones matmul (broadcast over partitions),
    batched 2 blocks per PSUM tile; fast reciprocal on DVE; Pool makes fp8
    normalized weights (wcn first — gates col pass — then wrn).
"""

import sys

sys.path.insert(0, "/opt/trn_rl_repo")

from contextlib import ExitStack

import numpy as np
import ml_dtypes

import concourse.bass as bass
import concourse.bacc as bacc
import concourse.tile as tile
from concourse import mybir
from concourse.bass_utils import run_bass_kernel_spmd

BF16 = mybir.dt.bfloat16
F32 = mybir.dt.float32
FP8 = mybir.dt.float8e4
AF = mybir.ActivationFunctionType
DR = mybir.MatmulPerfMode.DoubleRow
MUL = mybir.AluOpType.mult
ADD = mybir.AluOpType.add

N, C_IN, C_INNER, C_OUT, H, W = 8, 512, 64, 512, 96, 96
HW = H * W  # 9216
KC = C_IN // 128  # 4 contraction chunks

S0 = 16.0     # x scale (fp8)
STF = 256.0   # t/f weight scale
SM = 2048.0   # M weight scale
S1 = 64.0     # Gd (h) storage scale
S2 = 128.0    # normalized-weight scale; folded into the denominator "ones"
S3 = 64.0     # U / out storage scale

_cache = {}


def build_program():
    nc = bacc.Bacc()

    xq_d = nc.dram_tensor("x_q", (128, KC, HW), FP8, kind="ExternalInput")
    tfw_d = nc.dram_tensor("tf_wT", (128, KC, 128), FP8, kind="ExternalInput")
    mw_d = nc.dram_tensor("m_wT", (128, KC, C_OUT), FP8, kind="ExternalInput")
    tfb_d = nc.dram_tensor("tf_b", (128, 1), F32, kind="ExternalInput")
    mask_d = nc.dram_tensor("mask", (96, 96), BF16, kind="ExternalInput")
    ones_d = nc.dram_tensor("ones_s", (96, 128), BF16, kind="ExternalInput")
    out_d = nc.dram_tensor("out", (KC, 128, HW), FP8, kind="ExternalOutput")

    with ExitStack() as ctx:
        tc = ctx.enter_context(tile.TileContext(nc))
        p0 = ctx.enter_context(tc.tile_pool(name="p0", bufs=1))

        Xq = p0.tile([128, KC, H, W], FP8)
        ones_s = p0.tile([96, 128], BF16)  # value 1/S2
        mask = p0.tile([96, 96], BF16)
        mw = p0.tile([128, KC, C_OUT], FP8)

        nc.sync.dma_start(out=ones_s, in_=ones_d[:])
        nc.sync.dma_start(out=mask, in_=mask_d[:])
        nc.sync.dma_start(out=mw, in_=mw_d[:])
        xv = xq_d[:].rearrange("p a (h w) -> p a h w", h=H)

        TF = p0.tile([128, 2, H, W], BF16, tag="big", name="TF")
        T = TF[0:64, 0]
        F = TF[0:64, 1]

        Xflat = Xq.rearrange("p a h w -> p a (h w)")
        TFflat = TF.rearrange("p c h w -> p c (h w)")

        pgd = ctx.enter_context(tc.tile_pool(name="pgd", bufs=1, space="DRAM"))
        Gd = pgd.tile([H, W, C_OUT], FP8)  # h * S1, [y, x, c] (flat pixels)
        Gflat = Gd.rearrange("h w c -> (h w) c")

        with tc.tile_pool(name="pwr", bufs=1) as pwr, \
             tc.tile_pool(name="pwc", bufs=1) as pwc:
            Wr = pwr.tile([96, H, W], BF16)   # exp(row affinity)[i, y, x]
            Wc = pwc.tile([96, W, H], BF16)   # exp(col affinity)[j, x, y]
            wcn = pwc.tile([96, W, H], FP8)   # Wc * S2/D
            wrn = pwr.tile([96, H, W], FP8)   # Wr * S2/D

            hsc = S1 / (S0 * SM)
            NBP = HW // 256  # 36 conv block-pairs

            with tc.tile_pool(name="pgt", bufs=6) as pgt, \
                 tc.tile_pool(name="pg_ps", bufs=2, space="PSUM") as pg_ps:

                def emit_conv_bp(bp, drain):
                    """One pair of 128-pixel h-conv blocks -> Gd."""
                    gtb = pgt.tile([128, 2, C_OUT], FP8, tag="gt")
                    for half in range(2):
                        b = 2 * bp + half
                        psg = pg_ps.tile([128, C_OUT], F32, tag="pg")
                        for t in range(KC // 2):
                            nc.tensor.matmul(
                                psg,
                                Xflat[:, 2 * t:2 * t + 2,
                                      b * 128:(b + 1) * 128],
                                mw[:, 2 * t:2 * t + 2, :],
                                start=(t == 0), stop=(t == KC // 2 - 1),
                                perf_mode=DR)
                        if drain == "dve":
                            nc.vector.tensor_scalar_mul(gtb[:, half, :],
                                                        psg, hsc)
                        elif drain == "act":
                            nc.scalar.activation(gtb[:, half, :], psg,
                                                 AF.Copy, scale=hsc)
                        else:  # alternate
                            if half == 0:
                                nc.scalar.activation(gtb[:, half, :], psg,
                                                     AF.Copy, scale=hsc)
                            else:
                                nc.vector.tensor_scalar_mul(gtb[:, half, :],
                                                            psg, hsc)
                    gdv = Gflat[bp * 256:(bp + 1) * 256].rearrange(
                        "(a p) c -> p a c", a=2)
                    nc.scalar.dma_start(out=gdv, in_=gtb)

                # ---- phase 1: t/f conv (fp8 DR); x chunks loaded inline ----
                with tc.tile_pool(name="pe1", bufs=1) as pe1, \
                     tc.tile_pool(name="ptf_ps", bufs=2, space="PSUM") as ptf_ps:
                    tfw = pe1.tile([128, KC, 128], FP8)
                    tfb = pe1.tile([128, 1], F32)
                    nc.sync.dma_start(out=tfw, in_=tfw_d[:])
                    nc.sync.dma_start(out=tfb, in_=tfb_d[:])
                    tfbb = bass.AP(tensor=tfb.tensor, offset=tfb.offset,
                                   ap=[tfb.ap[0], [0, 512]])
                    for q in range(8):
                        nc.sync.dma_start(
                            out=Xq[:, :, q * 12:(q + 1) * 12, :],
                            in_=xv[:, :, q * 12:(q + 1) * 12, :])

                    for b in range(HW // 512):
                        sl = slice(b * 512, (b + 1) * 512)
                        pst = ptf_ps.tile([128, 512], F32, tag="pt")
                        for t in range(KC // 2):
                            nc.tensor.matmul(
                                pst, tfw[:, 2 * t:2 * t + 2, :],
                                Xflat[:, 2 * t:2 * t + 2, sl],
                                start=(t == 0), stop=(t == KC // 2 - 1),
                                perf_mode=DR)
                        nc.scalar.activation(TFflat[:, 0, sl], pst,
                                             AF.Identity, bias=tfb,
                                             scale=1.0 / (S0 * STF))
                        if b % 3 == 2:
                            sl3 = slice((b - 2) * 512, (b + 1) * 512)
                            nc.sync.dma_start(out=TFflat[0:64, 1, sl3],
                                              in_=TFflat[64:128, 0, sl3])

                # ---- phase 2: affinities + exp, conv bps interleaved ----
                # conv drains go to DVE here (Act is exp-bound).
                bp_next = 0
                with tc.tile_pool(name="pe_ps", bufs=2, space="PSUM") as pe_ps:
                    for y0 in range(0, H, 8):
                        ps = pe_ps.tile([96, 2, 512], F32, tag="pe")
                        for r in range(8):
                            nc.tensor.matmul(
                                ps[:, r // 4, (r % 4) * 96:(r % 4) * 96 + 96],
                                F[:, y0 + r, :], T[:, y0 + r, :],
                                start=True, stop=True)
                        nc.scalar.activation(
                            Wr[:, y0:y0 + 8, :].rearrange(
                                "i (a b) w -> i a (b w)", a=2),
                            ps[:, :, 0:384], AF.Exp)
                        emit_conv_bp(bp_next, "dve")
                        bp_next += 1
                    mb8 = bass.AP(tensor=mask.tensor, offset=mask.offset,
                                  ap=[mask.ap[0], [0, 8], mask.ap[1]])
                    for x0 in range(0, W, 8):
                        ps = pe_ps.tile([96, 2, 512], F32, tag="pe")
                        for r in range(8):
                            nc.tensor.matmul(
                                ps[:, r // 4, (r % 4) * 96:(r % 4) * 96 + 96],
                                F[:, :, x0 + r], T[:, :, x0 + r],
                                start=True, stop=True)
                        wcs = Wc[:, x0:x0 + 8, :]
                        nc.scalar.activation(
                            wcs.rearrange("j (a b) y -> j a (b y)", a=2),
                            ps[:, :, 0:384], AF.Exp)
                        nc.vector.tensor_mul(wcs, wcs, mb8)
                        emit_conv_bp(bp_next, "dve")
                        bp_next += 1

                WrT = Wr.rearrange("i h w -> i w h")
                WcT = Wc.rearrange("j x y -> j y x")

                # ---- phase 2.5: denominators, computed ONCE ([x, y] layout,
                # 2 blocks per PSUM tile) -> rrall; Pool normalizes wcn
                # (gates col pass) then wrn (transposed rrall view).
                # Remaining conv bps interleaved. ----
                rrall = p0.tile([128, W, H], F32)   # S2/D, broadcast, [x, y]
                rrT = rrall.rearrange("p x y -> p y x")
                with tc.tile_pool(name="pd_ps", bufs=2, space="PSUM") as pd_ps:
                    for b8 in range(W // 8):
                        psd = pd_ps.tile([128, 2, 512], F32, tag="pd")
                        for i in range(2):
                            s = slice(b8 * 8 + 4 * i, b8 * 8 + 4 * i + 4)
                            nc.tensor.matmul(psd[:, i, 0:384], ones_s,
                                             Wc[:, s, :],
                                             start=True, stop=False)
                            nc.tensor.matmul(psd[:, i, 0:384], ones_s,
                                             WrT[:, s, :],
                                             start=False, stop=True)
                        s8 = slice(b8 * 8, b8 * 8 + 8)
                        nc.vector.reciprocal_approx_fast(
                            rrall[:, s8, :].rearrange(
                                "p (a b) f -> p a (b f)", a=2),
                            psd[:, :, 0:384])
                        nc.gpsimd.tensor_mul(wcn[:, s8, :], Wc[:, s8, :],
                                             rrall[0:96, s8, :])
                        if bp_next < NBP:
                            emit_conv_bp(bp_next, "alt")
                            bp_next += 1
                    for b8 in range(H // 8):
                        s8 = slice(b8 * 8, b8 * 8 + 8)
                        nc.gpsimd.tensor_mul(wrn[:, s8, :], Wr[:, s8, :],
                                             rrT[0:96, s8, :])
                        if bp_next < NBP:
                            emit_conv_bp(bp_next, "alt")
                            bp_next += 1
                    while bp_next < NBP:
                        emit_conv_bp(bp_next, "alt")
                        bp_next += 1

            # ---- phases 4+5: col pass then row pass, one scope ----
            U = p0.tile([128, KC, H, W], FP8, tag="big", name="U")
            uo = 1.0 / (S1 * S2) * S3   # psum -> U-fp8 scale
            outv = out_d[:].rearrange("k p q -> p k q")
            with tc.tile_pool(name="pu1", bufs=4) as pu1, \
                 tc.tile_pool(name="pst2", bufs=3) as pst2, \
                 tc.tile_pool(name="pu_ps", bufs=4, space="PSUM") as pu_ps:
                for xb in range(W // 4):
                    x0 = xb * 4
                    xs = slice(x0, x0 + 4)
                    gcb = pu1.tile([96, 4, C_OUT], FP8, tag="gc")
                    nc.sync.dma_start(out=gcb, in_=Gd[:, xs, :])
                    for ch in range(2):
                        psu = pu_ps.tile([128, 2, 512], F32, tag="pu")
                        for c2 in range(2):
                            cc = 2 * ch + c2
                            for r in range(4):
                                nc.tensor.matmul(
                                    psu[:, c2, r * 96:(r + 1) * 96],
                                    gcb[:, r, cc * 128:(cc + 1) * 128],
                                    wcn[:, x0 + r, :],
                                    start=True, stop=True)
                        uv = U[:, 2 * ch:2 * ch + 2, :, xs]
                        psv = psu[:, :, 0:384].rearrange(
                            "p c (x y) -> p c y x", x=4)
                        if (2 * xb + ch) % 3 == 2:
                            nc.vector.tensor_scalar_mul(uv, psv, uo)
                        else:
                            nc.scalar.activation(uv, psv, AF.Copy, scale=uo)

                for yb in range(H // 4):
                    y0 = yb * 4
                    ys = slice(y0, y0 + 4)
                    rgb = pu1.tile([96, 4, C_OUT], FP8, tag="gc")
                    nc.sync.dma_start(
                        out=rgb, in_=Gd[ys].rearrange("y x c -> x y c"))
                    for ch in range(2):
                        psu = pu_ps.tile([128, 2, 512], F32, tag="pu")
                        for c2 in range(2):
                            cc = 2 * ch + c2
                            for r in range(4):
                                nc.tensor.matmul(
                                    psu[:, c2, r * 96:(r + 1) * 96],
                                    rgb[:, r, cc * 128:(cc + 1) * 128],
                                    wrn[:, y0 + r, :],
                                    start=True, stop=True)
                        uv = U[:, 2 * ch:2 * ch + 2, ys, :]
                        psv = psu[:, :, 0:384].rearrange(
                            "p c (a b) -> p c a b", a=4)
                        if (2 * yb + ch) % 3 == 2:
                            # Act scaled copy + Pool add (Act cannot add two
                            # tensors; Pool cannot read PSUM)
                            stg = pst2.tile([128, 2, 4, 96], BF16, tag="st")
                            nc.scalar.activation(stg, psv, AF.Copy, scale=uo)
                            nc.gpsimd.tensor_add(uv, uv, stg)
                        else:
                            nc.vector.scalar_tensor_tensor(
                                uv, psv, uo, uv, MUL, ADD)
                    if yb % 2 == 1:
                        sl8 = slice((yb - 1) * 4 * 96, (yb + 1) * 4 * 96)
                        nc.scalar.dma_start(
                            out=outv[:, :, sl8],
                            in_=U[:, :, (yb - 1) * 4:(yb + 1) * 4, :])

    nc.finalize()
    return nc


def _prep_shared(t_w, t_b, f_w, f_b, g_w, g_b, inc_w, inc_b):
    bf = ml_dtypes.bfloat16
    f8 = ml_dtypes.float8_e4m3
    tf_wT = np.concatenate([t_w.T, f_w.T], axis=1)  # (512, 128)
    M = inc_w @ g_w  # (512, 512)
    d = {
        "tf_wT": np.ascontiguousarray(
            (tf_wT * STF).reshape(KC, 128, 128).transpose(1, 0, 2)).astype(f8),
        "m_wT": np.ascontiguousarray(
            (M.T * SM).reshape(KC, 128, C_OUT).transpose(1, 0, 2)).astype(f8),
        "tf_b": np.concatenate([t_b, f_b]).reshape(128, 1).astype(np.float32),
        "mask": (1.0 - np.eye(96)).astype(bf),
        "ones_s": np.full((96, 128), 1.0 / S2, dtype=np.float32).astype(bf),
    }
    comb_b = inc_b + inc_w @ g_b
    return d, comb_b


def kernel(x, t_w, t_b, f_w, f_b, g_w, g_b, inc_w, inc_b):
    x = np.asarray(x, dtype=np.float32)
    shared, comb_b = _prep_shared(
        np.asarray(t_w, np.float32), np.asarray(t_b, np.float32),
        np.asarray(f_w, np.float32), np.asarray(f_b, np.float32),
        np.asarray(g_w, np.float32), np.asarray(g_b, np.float32),
        np.asarray(inc_w, np.float32), np.asarray(inc_b, np.float32))

    f8 = ml_dtypes.float8_e4m3
    in_maps = []
    for n in range(N):
        xi = x[n].reshape(KC, 128, HW)  # (4, 128, 9216)
        m = dict(shared)
        m["x_q"] = np.ascontiguousarray(
            xi.transpose(1, 0, 2) * S0).astype(f8)
        in_maps.append(m)

    if "nc" not in _cache:
        _cache["nc"] = build_program()
    res = run_bass_kernel_spmd(_cache["nc"], in_maps, core_ids=list(range(N)))
    attn = np.stack([r["out"].astype(np.float32).reshape(C_IN, H, W)
                     for r in res.results]) * (1.0 / S3)
    return x + attn + comb_b.astype(np.float32)[None, :, None, None]


if __name__ == "__main__":
    rng = np.random.default_rng(0)
    ins = {
        "x": rng.standard_normal((N, C_IN, H, W), dtype=np.float32),
        "t_w": rng.standard_normal((C_INNER, C_IN), dtype=np.float32) * 0.02,
        "t_b": np.zeros(C_INNER, np.float32),
        "f_w": rng.standard_normal((C_INNER, C_IN), dtype=np.float32) * 0.02,
        "f_b": np.zeros(C_INNER, np.float32),
        "g_w": rng.standard_normal((C_OUT, C_IN), dtype=np.float32) * 0.02,
        "g_b": np.zeros(C_OUT, np.float32),
        "inc_w": rng.standard_normal((C_IN, C_OUT), dtype=np.float32) * 0.02,
        "inc_b": np.zeros(C_IN, np.float32),
    }
    y = kernel(**ins)
    print(y.shape, y.dtype)
